# revision 41
# baseline (speedup 1.0000x reference)
"""GQA attention kernel for 8 TRN2 NeuronCores.

Problem: B=2, T=2048, C=4096, NH=32 q-heads, NKV=8 kv-heads, HD=128,
RoPE (theta=1e4), causal, f32 I/O.

Sharding: core = (batch b, kv-head-group g): b = core//4, g = core%4.
Each core owns batch b and kv heads {2g, 2g+1} (= q heads 8g..8g+7):
  - projects x[b] against its wq/wk/wv column slices (bf16 compute),
  - runs causal attention for its 8 q heads,
  - computes the partial o_proj x its wo row slice -> [T, C] f32.
Host sums the 4 partials per batch.

On-chip layout is feature-major ("X^T"): activations live as
[feature=partition, token=free] so every matmul contracts along
partitions. x is pre-transposed/bf16-cast on host; RoPE's rotate_half
is a 128x128 permutation matmul on the PE; softmax denominator comes
free from a ones-column appended to V.

Perf structure (vs the first working version):
  - warmup uses real matmuls (PE transposes don't engage the HAM clock
    gate, so the old warmup left the first ~20us at 1.2 GHz),
  - RoPE + the V transposes run inside the projection phase, chunk by
    chunk, so the attention phase is pure scores+PV on the PE,
  - exp is batched per [128,1024] two-tile strip (the ACTIVATE fixed
    overhead is ~352 cycles, so 512-wide exp wastes a third of ScalarE),
  - ScalarE does nothing but exp during attention; all PSUM evacuation
    there is on VectorE,
  - PV accumulators are packed 2-per-PSUM-bank (the odd group of a bank
    starts with start=False, relying on the even group's whole-bank
    has_written clear), leaving 6 banks for THREE double-buffered score
    strips -- the deep exp pipeline is what keeps the PE from ever
    stalling on ScalarE,
  - the diagonal causal mask is applied by zeroing the diagonal P block
    after exp (multiplicative 0/1 mask on the DVE) instead of masking S,
  - chunk-3's RoPE is injected into head 0's first attention strips,
    and the OT transposes are pipelined one token-tile ahead inside the
    o_proj matmul stream.
"""

import sys

sys.path.insert(0, "/opt/trn_rl_repo")

import numpy as np
import ml_dtypes

import concourse.bass as bass
import concourse.bacc as bacc
import concourse.mybir as mybir
import concourse.tile as tile
from concourse.bass_utils import run_bass_kernel_spmd

BF16 = mybir.dt.bfloat16
F32 = mybir.dt.float32
AF = mybir.ActivationFunctionType
ALU = mybir.AluOpType

B, T, C = 2, 2048, 4096
NH, NKV, HD = 32, 8, 128
THETA = 10000.0
NCORES = 8

QH = 8          # q heads per core
KV = 2          # kv heads per core
QC = 4          # token chunks of 512
KT = 16         # k tiles of 128
TT = 16         # token tiles of 128
CCH = 32        # contraction chunks of 128 over C

_CACHE = {}


def _build_nc():
    nc = bacc.Bacc("TRN2", target_bir_lowering=False, debug=False,
                   enable_asserts=False, num_devices=NCORES)

    xT_d = nc.dram_tensor("xT", [C, T], BF16, kind="ExternalInput")
    wqkv_d = nc.dram_tensor("wqkv", [CCH, 2, 128, 768], BF16, kind="ExternalInput")
    wo_d = nc.dram_tensor("wo", [QH * HD, C], BF16, kind="ExternalInput")
    cos_d = nc.dram_tensor("cosT", [128, T], BF16, kind="ExternalInput")
    sin_d = nc.dram_tensor("sinT", [128, T], BF16, kind="ExternalInput")
    prot_d = nc.dram_tensor("protT", [128, 128], BF16, kind="ExternalInput")
    ident_d = nc.dram_tensor("ident", [128, 128], BF16, kind="ExternalInput")
    cmask_d = nc.dram_tensor("cmask", [128, 128], BF16, kind="ExternalInput")
    out_d = nc.dram_tensor("out", [T, C], F32, kind="ExternalOutput")

    with tile.TileContext(nc) as tc:
        with tc.tile_pool(name="persist", bufs=1) as pp:
            ident = pp.tile([128, 128], BF16)
            nc.sync.dma_start(ident, ident_d.ap())
            cosT = pp.tile([128, T], BF16)
            sinT = pp.tile([128, T], BF16)
            prot = pp.tile([128, 128], BF16)
            cmask = pp.tile([128, 128], BF16)

            # HAM warm-up with REAL matmuls (transpose-mode doesn't count
            # as PE-busy for the clock gate) against a memset tile, so the
            # warm-up needs no DMA and starts immediately; it spans the
            # 3.4us SHORT window plus the first x/weight DMA latency, so
            # projections start at 2.4 GHz.
            warmsrc = pp.tile([128, 128], BF16)
            nc.vector.memset(warmsrc, 0.125)
            with tc.tile_pool(name="pwarm", bufs=2, space="PSUM") as pwp:
                for w in range(96):
                    wps = pwp.tile([128, 128], F32, name=f"warm{w % 2}",
                                   tag="warm")
                    nc.tensor.matmul(wps, warmsrc, warmsrc, start=True,
                                     stop=True)

            QT = pp.tile([128, QH, T], BF16)
            KTt = pp.tile([128, KV, T], BF16)
            VT = pp.tile([128, KV, T], BF16)
            OT = pp.tile([128, QH, T], BF16)
            Vn = pp.tile([128, KV, KT, 132], BF16)
            nc.vector.memset(Vn[:, :, :, 128:129], 1.0)

            # ------------- projections + fused RoPE / V-transpose ----------
            # Q^T/K^T/V^T = W^T @ x^T per 512-token chunk; as soon as a
            # chunk's heads are evacuated, RoPE (q' = q*cos + (P_rot@q)*sin)
            # and the V transpose for that chunk run, overlapped with the
            # next chunk's projection stream.
            with tc.tile_pool(name="xt", bufs=2) as xtp, \
                 tc.tile_pool(name="wt", bufs=8) as wtp, \
                 tc.tile_pool(name="pproj", bufs=6, space="PSUM") as ppj, \
                 tc.tile_pool(name="paux", bufs=2, space="PSUM") as pax, \
                 tc.tile_pool(name="ropes", bufs=3) as rsp:
                xview = xT_d.ap().rearrange("(c p) t -> p c t", p=128)

                def rope_chunk(src, qc):
                    # src: [128, T] slice view of one head; rotates chunk qc
                    rsl = slice(qc * 512, (qc + 1) * 512)
                    ps = pax.tile([128, 512], F32, tag="pax")
                    nc.tensor.matmul(ps, prot, src[:, rsl], start=True,
                                     stop=True)
                    rs = rsp.tile([128, 512], BF16, tag="rs")
                    nc.vector.tensor_tensor(rs, ps, sinT[:, rsl], op=ALU.mult)
                    nc.vector.tensor_tensor(src[:, rsl], src[:, rsl],
                                            cosT[:, rsl], op=ALU.mult)
                    nc.vector.tensor_tensor(src[:, rsl], src[:, rsl], rs,
                                            op=ALU.add)

                def vtrans_chunk(kv, qc):
                    # VT chunk [hd, 4x128 tok] -> Vn natural [tok, hd],
                    # 4 transposes into one PSUM bank, one DVE evacuation
                    pt = pax.tile([128, 4, 128], BF16, tag="pax")
                    for i in range(4):
                        kt = 4 * qc + i
                        nc.tensor.transpose(
                            pt[:, i, :], VT[:, kv, kt * 128:(kt + 1) * 128],
                            ident)
                    nc.vector.tensor_copy(Vn[:, kv, 4 * qc:4 * qc + 4, 0:128],
                                          pt)

                def fixup_chunk(qc, grp):
                    # RoPE + V-transpose for an ALREADY-FINISHED chunk,
                    # emitted mid-way through a later chunk's stream so the
                    # PE never waits on fresh evacuations for these.
                    if grp == 0:
                        for kv in range(KV):
                            rope_chunk(KTt[:, kv, :], qc)
                            vtrans_chunk(kv, qc)
                        for h in range(3):
                            rope_chunk(QT[:, h, :], qc)
                    else:
                        for h in range(3, QH):
                            rope_chunk(QT[:, h, :], qc)

                for qc in range(QC):
                    tsl = slice(qc * 512, (qc + 1) * 512)
                    xt = xtp.tile([128, CCH, 512], BF16)
                    # split the load (early c-chunks land first) and use the
                    # scalar HWDGE queue so weights stream in parallel on sync
                    for piece in range(4):
                        csl = slice(piece * 8, (piece + 1) * 8)
                        nc.scalar.dma_start(xt[:, csl, :], xview[:, csl, tsl])
                    if qc == 0:
                        # small constants on the software-DGE queue, emitted
                        # after the first weight stream so they don't compete
                        # for HBM during the critical projection ramp
                        nc.gpsimd.dma_start(cosT, cos_d.ap())
                        nc.gpsimd.dma_start(sinT, sin_d.ap())
                        nc.gpsimd.dma_start(prot, prot_d.ap())
                        nc.gpsimd.dma_start(cmask, cmask_d.ap())
                    for grp in range(2):
                        if qc > 0:
                            fixup_chunk(qc - 1, grp)
                        psums = [ppj.tile([128, 512], F32, name=f"pj{qc}_{grp}_{o}",
                                          tag="pj") for o in range(6)]
                        for c in range(CCH):
                            wt = wtp.tile([128, 768], BF16)
                            nc.sync.dma_start(wt, wqkv_d.ap()[c, grp])
                            for o in range(6):
                                nc.tensor.matmul(
                                    psums[o], wt[:, o * 128:(o + 1) * 128],
                                    xt[:, c, :], start=(c == 0), stop=(c == CCH - 1))
                        for o in range(6):
                            oi = grp * 6 + o
                            if oi < 8:
                                dst = QT[:, oi, tsl]
                            elif oi < 10:
                                dst = KTt[:, oi - 8, tsl]
                            else:
                                dst = VT[:, oi - 10, tsl]
                            # alternate engines so psum slots free faster
                            if o % 2 == 0:
                                nc.scalar.copy(dst, psums[o])
                            else:
                                nc.vector.tensor_copy(dst, psums[o])
                # chunk 3's RoPE / V-transpose is injected into the first
                # head's attention strips (late_acts below): head 0 only
                # touches chunk-3 K/Q from its 13th strip on, so the PE
                # ramps straight into attention instead of idling on the
                # last evacuations here.

            # wo load after the x^T/weight stream pools are gone, so it
            # overlaps attention without blowing SBUF
            wo_pool = tc.alloc_tile_pool(name="wop", bufs=1)
            wo_t = wo_pool.tile([128, QH, C], BF16)
            nc.sync.dma_start(wo_t, wo_d.ap().rearrange("(h p) n -> p h n", p=128))

            # ---------------- attention ------------------
            # S^T[k,q] = K @ Q^T in [128,1024] two-k-tile strips;
            # P^T = exp(S^T) one ACTIVATE per strip (the diagonal block of
            # P is zeroed afterwards on the DVE -- cheaper than masking S);
            # O = P @ [V|1].  Two heads are processed in lockstep so one
            # head's PE work hides the other head's exp latency.
            # PSUM: 2 strip bufs (4 banks) + 2 packed PV groups (4 banks).
            # po: one group per head in flight (4 tags x 1 buf = 4 banks);
            # the cross-group overlap comes from the partner head
            with tc.tile_pool(name="pst", bufs=2, space="PSUM") as stp, \
                 tc.tile_pool(name="po", bufs=1, space="PSUM") as pop, \
                 tc.tile_pool(name="pt", bufs=4) as ptp, \
                 tc.tile_pool(name="rc", bufs=4) as rcp:

                def strip(h, qc, p, pa, pb):
                    kv = h // 4
                    tsl = slice(qc * 512, (qc + 1) * 512)
                    st = stp.tile([128, 1024], F32, tag="st")
                    ptile = ptp.tile([128, 1024], BF16)
                    off = 0
                    diag = []
                    for half in range(2):
                        kt = 2 * p + half
                        ssl = slice(half * 512, (half + 1) * 512)
                        nc.tensor.matmul(
                            st[:, ssl], KTt[:, kv, kt * 128:(kt + 1) * 128],
                            QT[:, h, tsl], start=True, stop=True)
                        d = kt - 4 * qc
                        if d >= 0:
                            diag.append(slice(half * 512 + d * 128,
                                              half * 512 + (d + 1) * 128))
                            if half == 0:
                                off = d * 128
                    if off >= 256:
                        # late diagonal pair: the second tile's sub-diagonal
                        # region is large, so split the activation and skip it
                        s2 = 512 + off + 128
                        nc.scalar.activation(ptile[:, off:512], st[:, off:512],
                                             AF.Exp)
                        nc.scalar.activation(ptile[:, s2:], st[:, s2:], AF.Exp)
                    else:
                        nc.scalar.activation(ptile[:, off:], st[:, off:],
                                             AF.Exp)
                    for bsl in diag:
                        # causal mask: zero the above-diagonal part of the
                        # diagonal P block (cmask is the 0/1 lower-triangle)
                        nc.vector.tensor_tensor(ptile[:, bsl], ptile[:, bsl],
                                                cmask, op=ALU.mult)
                    for j in range(4):
                        qt = 4 * qc + j
                        for half in range(2):
                            kt = 2 * p + half
                            if kt <= qt:
                                # j=1/3 share a PSUM bank with j=0/2: the
                                # bank's has_written bits are cleared by the
                                # even group's kt=0 start, so the odd group
                                # must begin with start=False (cleared bits
                                # => overwrite+set, i.e. start semantics)
                                nc.tensor.matmul(
                                    (pa if j < 2 else pb)[:, j % 2, 0:129],
                                    ptile[:, half * 512 + j * 128:
                                          half * 512 + (j + 1) * 128],
                                    Vn[:, kv, kt, 0:129],
                                    start=(kt == 0 and j % 2 == 0),
                                    stop=(kt == qt),
                                    skip_group_check=(j % 2 == 1))

                def finish_group(h, qc, pa, pb):
                    # one strided reciprocal per packed pair of accumulators
                    rca = rcp.tile([128, 2], F32, tag="rca")
                    rcb = rcp.tile([128, 2], F32, tag="rcb")
                    nc.vector.reciprocal(rca, pa[:, :, 128:129])
                    nc.vector.reciprocal(rcb, pb[:, :, 128:129])
                    for j in range(4):
                        qt = 4 * qc + j
                        pj = (pa if j < 2 else pb)[:, j % 2, 0:129]
                        rc = (rca if j < 2 else rcb)[:, j % 2:j % 2 + 1]
                        # store O natural [tok, hd] into OT's block; the
                        # transposes interleaved into o_proj below fix the
                        # layout without stalling the PE mid-attention
                        nc.vector.tensor_scalar_mul(
                            OT[:, h, qt * 128:(qt + 1) * 128],
                            pj[:, 0:128], rc)

                # chunk-3 RoPE / V-transpose, deferred out of the projection
                # phase and injected into the attention ramp (their psum
                # lives in spare "st"-tag slots)
                def late_rope(src):
                    rsl = slice(3 * 512, 4 * 512)
                    ps = stp.tile([128, 512], F32, tag="st")
                    nc.tensor.matmul(ps, prot, src[:, rsl], start=True,
                                     stop=True)
                    rs = ptp.tile([128, 512], BF16, tag="lrs")
                    nc.vector.tensor_tensor(rs, ps, sinT[:, rsl], op=ALU.mult)
                    nc.vector.tensor_tensor(src[:, rsl], src[:, rsl],
                                            cosT[:, rsl], op=ALU.mult)
                    nc.vector.tensor_tensor(src[:, rsl], src[:, rsl], rs,
                                            op=ALU.add)

                def late_vtrans(kv):
                    pt = stp.tile([128, 4, 128], BF16, tag="st")
                    for i in range(4):
                        kt = 12 + i
                        nc.tensor.transpose(
                            pt[:, i, :], VT[:, kv, kt * 128:(kt + 1) * 128],
                            ident)
                    nc.vector.tensor_copy(Vn[:, kv, 12:16, 0:128], pt)

                late_acts = {
                    (0, 0): [lambda: late_vtrans(0)],
                    (1, 0): [lambda: late_rope(KTt[:, 0, :])],
                    (2, 0): [lambda: late_rope(KTt[:, 1, :])],
                    (3, 0): [lambda: late_vtrans(1)],
                    (4, 0): [lambda: late_rope(QT[:, 0, :])],
                    (5, 0): [lambda: late_rope(QT[:, 1, :])],
                    (6, 0): [lambda: late_rope(QT[:, 2, :])],
                    (7, 0): [lambda: late_rope(QT[:, 3, :])],
                    (8, 0): [lambda: late_rope(QT[:, 4, :])],
                    (9, 0): [lambda: late_rope(QT[:, 5, :])],
                    (10, 0): [lambda: late_rope(QT[:, 6, :])],
                    (11, 0): [lambda: late_rope(QT[:, 7, :])],
                }

                # head A walks qc ascending while its partner B walks qc
                # descending, interleaved strip-by-strip: a light strip of
                # one head pairs with a heavy strip of the other, so the
                # combined PE work per round stays level with the exp rate
                strips_asc = [(qc, p) for qc in range(QC)
                              for p in range(2 * qc + 2)]
                strips_desc = [(qc, p) for qc in reversed(range(QC))
                               for p in range(2 * qc + 2)]

                for hp in range(QH // 2):
                    accs = {}

                    def run(h, qc, p):
                        if p == 0:
                            accs[h] = (
                                pop.tile([128, 2, 130], F32,
                                         name=f"pa{h%2}_{qc}", tag=f"poa{h%2}"),
                                pop.tile([128, 2, 130], F32,
                                         name=f"pb{h%2}_{qc}", tag=f"pob{h%2}"),
                            )
                        strip(h, qc, p, *accs[h])
                        if p == 2 * qc + 1:
                            finish_group(h, qc, *accs[h])

                    for i in range(len(strips_asc)):
                        run(2 * hp, *strips_asc[i])
                        if hp == 0:
                            for f in late_acts.get((i, 0), []):
                                f()
                        run(2 * hp + 1, *strips_desc[i])
                        if hp == 0:
                            for f in late_acts.get((i, 1), []):
                                f()

            # ---------------- o_proj partial: O @ wo_slice ----------------
            with tc.tile_pool(name="pout", bufs=6, space="PSUM") as outp, \
                 tc.tile_pool(name="potr", bufs=2, space="PSUM") as otrp, \
                 tc.tile_pool(name="ostg", bufs=6) as stgp:
                # in-place transposes OT [tok,hd] -> [hd,tok], pipelined one
                # token-tile ahead of the matmul stream so the PE never
                # waits on the DVE copy-backs
                def otrans(qt):
                    osl = slice(qt * 128, (qt + 1) * 128)
                    for h in range(QH):
                        ptr = otrp.tile([128, 128], BF16,
                                        name=f"otr{h}_{qt}", tag="otr")
                        nc.tensor.transpose(ptr, OT[:, h, osl], ident)
                        nc.vector.tensor_copy(OT[:, h, osl], ptr)

                otrans(0)
                for tt in range(TT):
                    if tt + 1 < TT:
                        otrans(tt + 1)
                    psl = slice(tt * 128, (tt + 1) * 128)
                    for n in range(8):
                        nsl = slice(n * 512, (n + 1) * 512)
                        ps = outp.tile([128, 512], F32)
                        for h in range(QH):
                            nc.tensor.matmul(ps, OT[:, h, psl],
                                             wo_t[:, h, nsl],
                                             start=(h == 0), stop=(h == QH - 1))
                        stg = stgp.tile([128, 512], F32)
                        # alternate engines: halves the serial evacuation
                        # chain, which sets the end-of-kernel store tail
                        if n % 2 == 0:
                            nc.scalar.copy(stg, ps)
                        else:
                            nc.vector.tensor_copy(stg, ps)
                        nc.sync.dma_start(out_d.ap()[psl, nsl], stg)

            wo_pool.release()

    nc.compile()
    return nc


def _host_prep(x, wq, wk, wv, wo):
    bf = ml_dtypes.bfloat16
    scale = HD ** -0.5

    # RoPE tables, feature-major [128, T]
    inv_freq = 1.0 / (THETA ** (np.arange(0, HD, 2, dtype=np.float32) / HD))
    t = np.arange(T, dtype=np.float32)
    freqs = np.outer(t, inv_freq)                      # [T, 64]
    emb = np.concatenate([freqs, freqs], -1)           # [T, 128]
    cosT = np.ascontiguousarray(np.cos(emb).T).astype(bf)
    sinT = np.ascontiguousarray(np.sin(emb).T).astype(bf)

    # rotate_half as a permutation matrix, pre-transposed for lhsT:
    # rot = P_rot @ q with P_rot[i, i+64] = -1 (i<64), P_rot[i, i-64] = +1.
    protT = np.zeros((128, 128), np.float32)
    for i in range(64):
        protT[i + 64, i] = -1.0
        protT[i, i + 64] = 1.0
    protT = protT.astype(bf)

    ident = np.eye(128, dtype=np.float32).astype(bf)

    # multiplicative causal mask for the diagonal [128k, 128q] block of P
    # (identical for every diagonal tile: valid iff q_local >= k_local)
    kl = np.arange(128)[:, None]
    ql = np.arange(128)[None, :]
    cmask = np.ascontiguousarray(
        np.where(ql >= kl, 1.0, 0.0).astype(np.float32)).astype(bf)

    xT = []
    for b in range(B):
        xT.append(np.ascontiguousarray(x[b].astype(bf).T))

    wqkv, wob = [], []
    for g in range(4):
        q_s = (wq[:, g * 1024:(g + 1) * 1024] * scale).astype(bf)
        k_s = wk[:, g * 256:(g + 1) * 256].astype(bf)
        v_s = wv[:, g * 256:(g + 1) * 256].astype(bf)
        wall = np.concatenate([q_s, k_s, v_s], axis=1)       # [C, 1536]
        wall = wall.reshape(CCH, 128, 2, 768).transpose(0, 2, 1, 3)
        wqkv.append(np.ascontiguousarray(wall))              # [32, 2, 128, 768]
        wob.append(np.ascontiguousarray(
            wo[g * 1024:(g + 1) * 1024, :].astype(bf)))      # [1024, C]

    in_maps = []
    for core in range(NCORES):
        b, g = core // 4, core % 4
        in_maps.append({
            "xT": xT[b], "wqkv": wqkv[g], "wo": wob[g],
            "cosT": cosT, "sinT": sinT, "protT": protT,
            "ident": ident, "cmask": cmask,
        })
    return in_maps


def kernel(x, wq, wk, wv, wo, _trace=False, _tmpdir=None):
    if "nc" not in _CACHE:
        _CACHE["nc"] = _build_nc()
    nc = _CACHE["nc"]

    in_maps = _host_prep(x, wq, wk, wv, wo)
    res = run_bass_kernel_spmd(nc, in_maps, core_ids=list(range(NCORES)),
                               trace=_trace, tmpdir=_tmpdir)
    _CACHE["last_results"] = res

    out = np.zeros((B, T, C), np.float32)
    for core in range(NCORES):
        out[core // 4] += res.results[core]["out"]
    return out
